# revision 16
# baseline (speedup 1.0000x reference)
"""Trainium2 Bass kernel for online forward-mode sensitivity propagation.

Math restructuring: the tangent recurrence for parameter direction p=(i,j)
is linear in (jx, jv) with forcing x_t[n,j] * e_i injected each step.  By
superposition over injection times,

    jac[n,d,i,j] = sum_t Kx(T-1-t)[i,d] * x_t[n,j]

where Kx(tau) is the impulse-response propagator: Kx(0)=dt^2*I, Kv(0)=dt*I,
Kv' = Kv + dt*Kx@(W^T - I), Kx' = Kx + dt*Kv'.  Working with Mx = Kx^T the
propagator recurrence becomes *identical* to the primal state recurrence
(left-multiplied by (W - I)), so one combined 128-row state
Z = [x^T | Mx ; v^T | Mv] advances with a single [128,80] matmul per step:
Z' = M @ Z.  The kernel squares M once and runs TWO interleaved
double-step chains (even/odd), halving the serial dependency length.
The Jacobian then factorizes as, per sample row n,

    jac[n]  (as a [(d,i), j] = [4096, 64] matrix)  =  G @ H_n

with G[(d,i), t] = Mx(T-1-t)[d,i] and H_n[t, j] = x_t[n, j] — a rank-16
contraction instead of propagating 4096 tangent states for 16 steps.

Big-stage implementation notes:
- Operands are split hi/lo in bf16 and stacked along the contraction dim
  (K = 4*T = 64 rows: Gh|Gl|Gh|Gl against Xh|Xh|Xl|Xl), so one bf16 matmul
  computes the full (Gh+Gl)^T (Xh+Xl) product with fp32 PSUM accumulation
  (~1e-7 operand error) — ~4x faster than the PE's fp32 LOW_HIGH mode.
- Outputs are produced in (d,i)-row pairs: even rows via one matmul into
  bank A, odd rows via a second matmul into bank B, run CONCURRENTLY in
  different PE row-groups (tile_position (0,0)/(64,0) with replicated
  operands).  A single strided DVE copy interleaves the pair so the store
  DMA gets 512-byte contiguous runs; the store stream runs at the
  HBM-per-core roofline (~350 GB/s).

Sharding: the 128 sample rows are split 16-per-core across 8 NeuronCores
(rows of x evolve independently); each core computes its own (replicated,
tiny) propagator chain and its 16.8 MB Jacobian shard.
"""

import numpy as np
from contextlib import ExitStack

DT = 0.01
NCORES = 8
N, D = 128, 64  # problem dims (hardcoded per spec)


def _build_nc(T, SE, NL):
    import concourse.bass as bass
    import concourse.tile as tile
    from concourse import bacc, mybir

    f32 = mybir.dt.float32
    bf16 = mybir.dt.bfloat16
    i32 = mybir.dt.int32
    dt = DT
    NF = T // SE + 1            # stored trajectory frames (incl. x0 and x_T)
    PB = T * NL                 # total (t, n) rows for the X reshape
    NB = 4                      # number of transpose blocks
    BS = PB // NB               # Y columns per block
    NH = (NL * D) // 512        # big-matmul free-dim halves
    NPH = 512 // D              # samples per free-dim half
    NG = (D * D) // 256         # 256-row output chunks
    TPB = T // NB               # chain steps per transpose block (= 4)
    K = 4 * T                   # stacked contraction length
    SD = NL + D                 # chain state free width (80)

    nc = bacc.Bacc(
        "TRN2",
        target_bir_lowering=False,
        debug=False,
        enable_asserts=False,
        num_devices=NCORES,
    )

    x0s = nc.dram_tensor("x0s", [NL, D], f32, kind="ExternalInput").ap()
    id_in = nc.dram_tensor("ident", [128, 128], f32, kind="ExternalInput").ap()
    v0s = nc.dram_tensor("v0s", [NL, D], f32, kind="ExternalInput").ap()
    w_in = nc.dram_tensor("w", [D, D], f32, kind="ExternalInput").ap()
    jac_out = nc.dram_tensor("jac", [NL, D * D, D], f32, kind="ExternalOutput").ap()
    traj_out = nc.dram_tensor("traj", [NF, NL, D], f32, kind="ExternalOutput").ap()

    with tile.TileContext(nc) as tc, ExitStack() as ctx:
        const = ctx.enter_context(tc.tile_pool(name="const", bufs=1))
        chain = ctx.enter_context(tc.tile_pool(name="chain", bufs=6))
        acc = ctx.enter_context(tc.tile_pool(name="acc", bufs=1))
        psA = ctx.enter_context(tc.tile_pool(name="psA", bufs=3, space="PSUM"))
        psT = ctx.enter_context(tc.tile_pool(name="psT", bufs=1, space="PSUM"))
        psB = ctx.enter_context(tc.tile_pool(name="psB", bufs=2, space="PSUM"))
        osb = ctx.enter_context(tc.tile_pool(name="osb", bufs=4))
        dram = ctx.enter_context(tc.tile_pool(name="dram", bufs=1, space="DRAM"))

        mult = mybir.AluOpType.mult
        add = mybir.AluOpType.add

        # ---- setup: load inputs; identity built on DVE (gpsimd starts late) ----
        xin = const.tile([NL, D], f32)
        nc.sync.dma_start(xin[:], x0s)
        vin = const.tile([NL, D], f32)
        nc.sync.dma_start(vin[:], v0s)
        wsb = const.tile([D, D], f32)
        nc.sync.dma_start(wsb[:], w_in)
        ident = const.tile([128, 128], f32)
        nc.sync.dma_start(ident[:], id_in)
        idA = ident[0:D, 0:D]            # I64 on partitions 0..63
        idB = ident[D:128, D:128]        # I64 on partitions 64..127

        ps_wt = psT.tile([D, D], f32, tag="pst")
        nc.tensor.transpose(ps_wt[:], wsb[:], idA)
        wmi = const.tile([D, D], f32)  # B = W^T - I
        nc.vector.scalar_tensor_tensor(wmi[:], idA, -1.0, ps_wt[:], op0=mult, op1=add)

        # ---- step matrix (transposed): Mt = M^T = [[I+dt^2 B, dt B],[dt I, I]] ----
        Mt = const.tile([128, 128], f32)
        nc.vector.scalar_tensor_tensor(
            Mt[0:D, 0:D], wmi[:], dt * dt, idA, op0=mult, op1=add
        )
        nc.scalar.mul(Mt[0:D, D:128], wmi[:], dt)
        nc.scalar.mul(Mt[D:128, 0:D], idB, dt)
        nc.scalar.copy(Mt[D:128, D:128], idB)

        # M2t = (M^2)^T = M^T M^T  via  matmul(lhsT=M, rhs=Mt)
        ps_m = psT.tile([128, 128], f32, tag="pst")
        nc.tensor.transpose(ps_m[:], Mt[:], ident[:])
        Msb = const.tile([128, 128], f32)
        nc.vector.tensor_copy(Msb[:], ps_m[:])
        ps_m2 = psT.tile([128, 128], f32, tag="pst")
        nc.tensor.matmul(ps_m2[:], Msb[:], Mt[:], start=True, stop=True)
        M2t = const.tile([128, 128], f32)
        nc.vector.tensor_copy(M2t[:], ps_m2[:])
        ps_m2b = psT.tile([128, 128], f32, tag="pst")
        nc.tensor.matmul(ps_m2b[:], Mt[:], Msb[:], start=True, stop=True)
        M2 = const.tile([128, 128], f32)
        nc.scalar.copy(M2[:], ps_m2b[:])
        ps_m4 = psT.tile([128, 128], f32, tag="pst")
        nc.tensor.matmul(ps_m4[:], M2[:], M2t[:], start=True, stop=True)
        M4t = const.tile([128, 128], f32)
        nc.vector.tensor_copy(M4t[:], ps_m4[:])

        # ---- initial state Z0 = [x0^T | dt^2 I ; v0^T | dt I] ----
        Z0 = chain.tile([128, SD], f32, tag="Z")
        XV = const.tile([NL, 2 * D], f32)
        nc.vector.tensor_copy(XV[:, 0:D], xin[:])
        nc.vector.tensor_copy(XV[:, D:2 * D], vin[:])
        ps_z = psT.tile([128, NL], f32, tag="pst")
        nc.tensor.matmul(ps_z[:], XV[:], ident[0:NL, 0:NL], start=True, stop=True)
        nc.vector.tensor_copy(Z0[:, 0:NL], ps_z[:])
        nc.scalar.mul(Z0[0:D, NL:SD], idA, dt * dt)
        nc.scalar.mul(Z0[D:128, NL:SD], idB, dt)

        # ---- accumulators + scratch ----
        Y = acc.tile([D, PB], f32)            # Y[:, t*NL+n] = x_t^T column n
        G_acc = acc.tile([D, T * D], f32)     # col block t*D = G row t = Mx(T-1-t)
        Gh = acc.tile([D, T * D], bf16)
        Gr = acc.tile([D, T * D], f32)
        Gl = acc.tile([D, T * D], bf16)
        Gst = [
            dram.tile([2, T // 2, D, D], bf16, name=f"gs{th}", tag=f"gs{th}")
            for th in range(2)
        ]
        Xst = [
            dram.tile([2, 2, TPB, NL, D], bf16, name=f"xs{b}", tag=f"xs{b}")
            for b in range(NB)
        ]
        GfatA = acc.tile([128, D * D], bf16)
        GfatB = acc.tile([128, D * D], bf16)
        XfatA = acc.tile([128, NL * D], bf16)
        XfatB = acc.tile([128, NL * D], bf16)
        rr = [nc.sync, nc.scalar]

        def extract(tau, Z):
            """Pull x_tau^T and Mx(tau) out of state Z (off the DVE queue)."""
            if tau % 2 == 0:
                nc.gpsimd.tensor_copy(Y[:, tau * NL:(tau + 1) * NL], Z[0:D, 0:NL])
                nc.gpsimd.tensor_copy(
                    G_acc[:, (T - 1 - tau) * D:(T - tau) * D], Z[0:D, NL:SD]
                )
            else:
                nc.scalar.copy(Y[:, tau * NL:(tau + 1) * NL], Z[0:D, 0:NL])
                nc.scalar.copy(
                    G_acc[:, (T - 1 - tau) * D:(T - tau) * D], Z[0:D, NL:SD]
                )

        def emit_g_half(th):
            """hi/lo-split + scratch store + stack-read of G rows th*T/2 ...."""
            c0, c1 = th * (T // 2) * D, (th + 1) * (T // 2) * D
            nc.vector.tensor_copy(Gh[:, c0:c1], G_acc[:, c0:c1])
            nc.vector.scalar_tensor_tensor(
                Gr[:, c0:c1], Gh[:, c0:c1], -1.0, G_acc[:, c0:c1], op0=mult, op1=add
            )
            nc.vector.tensor_copy(Gl[:, c0:c1], Gr[:, c0:c1])
            for hl, src in ((0, Gh), (1, Gl)):
                rr[hl].dma_start(
                    Gst[th][hl].rearrange("t d i -> d t i"),
                    src[:, c0:c1].rearrange("d (t i) -> d t i", t=T // 2),
                )
            # stack row k = th*32 + q2*16 + hl*8 + tt; G row = G_hl[t], dup in q2
            gsrc = Gst[th][:].rearrange("hl t d i -> (hl t) (d i)")
            m = th
            for rep, fat in ((0, GfatA), (1, GfatB)):
                for q2 in range(2):
                    r0 = rep * K + th * 32 + q2 * 16
                    rr[m % 2].dma_start(fat[r0:r0 + 16, :], gsrc)
                    m += 1

        def emit_block_reshape(b):
            """Transpose Y block b -> natural x rows; traj + hi/lo stack rows."""
            ps_tb = psT.tile([BS, D], f32, tag="pst")
            nc.tensor.transpose(ps_tb[:], Y[:, b * BS:(b + 1) * BS], idA)
            tb = osb.tile([BS, D], f32, tag="tb_sb")
            nc.scalar.copy(tb[:], ps_tb[:])
            for f in range(NF - 1):
                t = f * SE
                if b * BS <= t * NL < (b + 1) * BS:
                    off = t * NL - b * BS
                    nc.sync.dma_start(traj_out[f], tb[off:off + NL, :])
            tbh = osb.tile([BS, D], bf16, tag="tbh")
            nc.vector.tensor_copy(tbh[:], tb[:])
            tbr = osb.tile([BS, D], f32, tag="tbr")
            nc.vector.scalar_tensor_tensor(
                tbr[:], tbh[:], -1.0, tb[:], op0=mult, op1=add
            )
            tbl = osb.tile([BS, D], bf16, tag="tbl")
            nc.vector.tensor_copy(tbl[:], tbr[:])
            m = b
            for xhl, srct in ((0, tbh), (1, tbl)):
                for dup in range(2):
                    rr[m % 2].dma_start(
                        Xst[b][xhl, dup].rearrange("t n j -> (t n) j"), srct[:]
                    )
                    m += 1
            # stack row k = th*32 + q2*16 + dup*8 + ttg; X row = X_q2[t], G row = G_dup[t]
            th, qq = b // 2, b % 2
            for rep, fat in ((0, XfatA), (1, XfatB)):
                for q2 in range(2):
                    for dup in range(2):
                        r0 = rep * K + th * 32 + q2 * 16 + dup * 8 + qq * TPB
                        rr[m % 2].dma_start(
                            fat[r0:r0 + TPB, :],
                            Xst[b][q2, dup].rearrange("t n j -> t (n j)"),
                        )
                        m += 1

        # ---- the chain: two interleaved double-step sequences ----
        Zs = [None] * (T + 1)
        Zs[0] = Z0
        extract(0, Z0)
        ps1 = psA.tile([128, SD], f32, tag="psz")
        nc.tensor.matmul(ps1[:], Mt[:], Z0[:], start=True, stop=True)
        Z1 = chain.tile([128, SD], f32, tag="Z")
        nc.vector.tensor_copy(Z1[:], ps1[:])
        Zs[1] = Z1
        extract(1, Z1)
        for tau in range(2, T + 1):
            ps = psA.tile([128, SD], f32, tag="psz")
            if tau == 2:
                nc.tensor.matmul(ps[:], M2t[:], Zs[0][:], start=True, stop=True)
            elif tau == 3:
                nc.tensor.matmul(ps[:], M2t[:], Zs[1][:], start=True, stop=True)
            else:
                nc.tensor.matmul(ps[:], M4t[:], Zs[tau - 4][:], start=True, stop=True)
            Z = chain.tile([128, SD], f32, tag="Z")
            if tau % 2 == 0:
                nc.vector.tensor_copy(Z[:], ps[:])
            else:
                nc.scalar.copy(Z[:], ps[:])
            Zs[tau] = Z
            if tau < T:
                extract(tau, Z)
            if (tau + 1) % TPB == 0:
                emit_block_reshape((tau + 1) // TPB - 1)
            if tau == T // 2:
                emit_g_half(1)   # G rows T/2..T-1 came from chain steps 0..T/2-1
        emit_g_half(0)

        # final frame x_T from the last chain state
        ps_xT = psT.tile([NL, D], f32, tag="pst")
        nc.tensor.transpose(ps_xT[:], Zs[T][0:D, 0:NL], idA)
        xT_sb = osb.tile([NL, D], f32, tag="xT_sb")
        nc.scalar.copy(xT_sb[:], ps_xT[:])
        nc.sync.dma_start(traj_out[NF - 1], xT_sb[:])

        # ---- big stage: paired even/odd chunks in concurrent row groups ----
        jac_pair = jac_out.rearrange("n (gq p2) j -> gq n (p2 j)", p2=2)
        m = 0
        for g in range(NG):
            for h in range(NH):
                hs = slice(h * 512, (h + 1) * 512)
                ps = psB.tile([128, 1024], f32)
                nc.tensor.matmul(
                    ps[:, 0:512],
                    GfatA[0:K, g * 256:(g + 1) * 256:2],
                    XfatA[0:K, hs],
                    start=True, stop=True,
                )
                nc.tensor.matmul(
                    ps[:, 512:1024],
                    GfatB[K:2 * K, g * 256 + 1:(g + 1) * 256:2],
                    XfatB[K:2 * K, hs],
                    start=True, stop=True,
                    tile_position=(64, 0),
                )
                ot = osb.tile([128, 1024], f32, tag="ot")
                ov = ot[:].rearrange("q (n p2 j) -> q n p2 j", n=NPH, p2=2)
                iv = ps[:].rearrange("q (p2 n j) -> q n p2 j", p2=2, n=NPH)
                if m % 3 == 2:
                    nc.scalar.copy(ov, iv)
                else:
                    nc.vector.tensor_copy(ov, iv)
                nc.sync.dma_start(
                    jac_pair[g * 128:(g + 1) * 128, h * NPH:(h + 1) * NPH, :],
                    ot[:].rearrange("q (n p2 j) -> q n (p2 j)", n=NPH, p2=2),
                )
                m += 1

    nc.compile()
    return nc


_cache = {}


def _get_nc(T, SE, NL):
    key = (T, SE, NL)
    if key not in _cache:
        _cache[key] = _build_nc(T, SE, NL)
    return _cache[key]


def kernel(x0, v0, force_weight, n_steps, store_every, _trace=False):
    from concourse.bass_utils import run_bass_kernel_spmd

    x0 = np.asarray(x0, dtype=np.float32)
    v0 = np.asarray(v0, dtype=np.float32)
    W = np.asarray(force_weight, dtype=np.float32)
    T = int(n_steps)
    SE = int(store_every)
    n, d = x0.shape
    assert (n, d) == (N, D)
    NL = n // NCORES
    NF = T // SE + 1

    nc = _get_nc(T, SE, NL)
    in_maps = [
        {
            "x0s": np.ascontiguousarray(x0[k * NL:(k + 1) * NL]),
            "v0s": np.ascontiguousarray(v0[k * NL:(k + 1) * NL]),
            "w": np.ascontiguousarray(W),
            "ident": np.eye(128, dtype=np.float32),
        }
        for k in range(NCORES)
    ]
    res = run_bass_kernel_spmd(
        nc, in_maps, core_ids=list(range(NCORES)), trace=_trace
    )
    kernel.last_results = res

    jac = np.concatenate(
        [res.results[k]["jac"].reshape(NL, D, D, D) for k in range(NCORES)], axis=0
    )
    traj = np.concatenate(
        [res.results[k]["traj"] for k in range(NCORES)], axis=1
    )
    assert traj.shape == (NF, n, d) and jac.shape == (n, d, D, D)
    return traj, jac


# revision 18
# speedup vs baseline: 1.0150x; 1.0150x over previous
"""Trainium2 Bass kernel for online forward-mode sensitivity propagation.

Math restructuring: the tangent recurrence for parameter direction p=(i,j)
is linear in (jx, jv) with forcing x_t[n,j] * e_i injected each step.  By
superposition over injection times,

    jac[n,d,i,j] = sum_t Kx(T-1-t)[i,d] * x_t[n,j]

where Kx(tau) is the impulse-response propagator: Kx(0)=dt^2*I, Kv(0)=dt*I,
Kv' = Kv + dt*Kx@(W^T - I), Kx' = Kx + dt*Kv'.  Working with Mx = Kx^T the
propagator recurrence becomes *identical* to the primal state recurrence
(left-multiplied by (W - I)), so one combined 128-row state
Z = [x^T | Mx ; v^T | Mv] advances with a single [128,80] matmul per step:
Z' = M @ Z.  The kernel squares M once and runs TWO interleaved
double-step chains (even/odd), halving the serial dependency length.
The Jacobian then factorizes as, per sample row n,

    jac[n]  (as a [(d,i), j] = [4096, 64] matrix)  =  G @ H_n

with G[(d,i), t] = Mx(T-1-t)[d,i] and H_n[t, j] = x_t[n, j] — a rank-16
contraction instead of propagating 4096 tangent states for 16 steps.

Big-stage implementation notes:
- Operands are split hi/lo in bf16 and stacked along the contraction dim
  (K = 4*T = 64 rows: Gh|Gl|Gh|Gl against Xh|Xh|Xl|Xl), so one bf16 matmul
  computes the full (Gh+Gl)^T (Xh+Xl) product with fp32 PSUM accumulation
  (~1e-7 operand error) — ~4x faster than the PE's fp32 LOW_HIGH mode.
- Outputs are produced in (d,i)-row pairs: even rows via one matmul into
  bank A, odd rows via a second matmul into bank B, run CONCURRENTLY in
  different PE row-groups (tile_position (0,0)/(64,0) with replicated
  operands).  A single strided DVE copy interleaves the pair so the store
  DMA gets 512-byte contiguous runs; the store stream runs at the
  HBM-per-core roofline (~350 GB/s).

Sharding: the 128 sample rows are split 16-per-core across 8 NeuronCores
(rows of x evolve independently); each core computes its own (replicated,
tiny) propagator chain and its 16.8 MB Jacobian shard.
"""

import numpy as np
from contextlib import ExitStack

DT = 0.01
NCORES = 8
N, D = 128, 64  # problem dims (hardcoded per spec)


def _build_nc(T, SE, NL):
    import concourse.bass as bass
    import concourse.tile as tile
    from concourse import bacc, mybir

    f32 = mybir.dt.float32
    bf16 = mybir.dt.bfloat16
    i32 = mybir.dt.int32
    dt = DT
    NF = T // SE + 1            # stored trajectory frames (incl. x0 and x_T)
    PB = T * NL                 # total (t, n) rows for the X reshape
    NB = 2                      # number of transpose blocks
    BS = PB // NB               # Y columns per block
    NH = (NL * D) // 512        # big-matmul free-dim halves
    NPH = 512 // D              # samples per free-dim half
    NG = (D * D) // 256         # 256-row output chunks
    TPB = T // NB               # chain steps per transpose block (= 4)
    K = 4 * T                   # stacked contraction length
    SD = NL + D                 # chain state free width (80)

    nc = bacc.Bacc(
        "TRN2",
        target_bir_lowering=False,
        debug=False,
        enable_asserts=False,
        num_devices=NCORES,
    )

    x0s = nc.dram_tensor("x0s", [NL, D], f32, kind="ExternalInput").ap()
    id_in = nc.dram_tensor("ident", [128, 128], f32, kind="ExternalInput").ap()
    v0s = nc.dram_tensor("v0s", [NL, D], f32, kind="ExternalInput").ap()
    w_in = nc.dram_tensor("w", [D, D], f32, kind="ExternalInput").ap()
    jac_out = nc.dram_tensor("jac", [NL, D * D, D], f32, kind="ExternalOutput").ap()
    traj_out = nc.dram_tensor("traj", [NF, NL, D], f32, kind="ExternalOutput").ap()

    with tile.TileContext(nc) as tc, ExitStack() as ctx:
        const = ctx.enter_context(tc.tile_pool(name="const", bufs=1))
        chain = ctx.enter_context(tc.tile_pool(name="chain", bufs=6))
        acc = ctx.enter_context(tc.tile_pool(name="acc", bufs=1))
        psA = ctx.enter_context(tc.tile_pool(name="psA", bufs=3, space="PSUM"))
        psT = ctx.enter_context(tc.tile_pool(name="psT", bufs=1, space="PSUM"))
        psB = ctx.enter_context(tc.tile_pool(name="psB", bufs=2, space="PSUM"))
        osb = ctx.enter_context(tc.tile_pool(name="osb", bufs=5))
        dram = ctx.enter_context(tc.tile_pool(name="dram", bufs=1, space="DRAM"))

        mult = mybir.AluOpType.mult
        add = mybir.AluOpType.add

        # ---- setup: load inputs; identity built on DVE (gpsimd starts late) ----
        xin = const.tile([NL, D], f32)
        nc.sync.dma_start(xin[:], x0s)
        vin = const.tile([NL, D], f32)
        nc.scalar.dma_start(vin[:], v0s)
        wsb = const.tile([D, D], f32)
        nc.scalar.dma_start(wsb[:], w_in)
        ident = const.tile([128, 128], f32)
        nc.sync.dma_start(ident[:], id_in)
        idA = ident[0:D, 0:D]            # I64 on partitions 0..63
        idB = ident[D:128, D:128]        # I64 on partitions 64..127

        ps_wt = psT.tile([D, D], f32, tag="pst")
        nc.tensor.transpose(ps_wt[:], wsb[:], idA)
        wmi = const.tile([D, D], f32)  # B = W^T - I
        nc.vector.scalar_tensor_tensor(wmi[:], idA, -1.0, ps_wt[:], op0=mult, op1=add)

        # ---- step matrix (transposed): Mt = M^T = [[I+dt^2 B, dt B],[dt I, I]] ----
        Mt = const.tile([128, 128], f32)
        nc.vector.scalar_tensor_tensor(
            Mt[0:D, 0:D], wmi[:], dt * dt, idA, op0=mult, op1=add
        )
        nc.scalar.mul(Mt[0:D, D:128], wmi[:], dt)
        nc.scalar.mul(Mt[D:128, 0:D], idB, dt)
        nc.scalar.copy(Mt[D:128, D:128], idB)

        # M2t = (M^2)^T = M^T M^T  via  matmul(lhsT=M, rhs=Mt)
        ps_m = psT.tile([128, 128], f32, tag="pst")
        nc.tensor.transpose(ps_m[:], Mt[:], ident[:])
        Msb = const.tile([128, 128], f32)
        nc.vector.tensor_copy(Msb[:], ps_m[:])
        ps_m2 = psT.tile([128, 128], f32, tag="pst")
        nc.tensor.matmul(ps_m2[:], Msb[:], Mt[:], start=True, stop=True)
        M2t = const.tile([128, 128], f32)
        nc.vector.tensor_copy(M2t[:], ps_m2[:])
        ps_m2b = psT.tile([128, 128], f32, tag="pst")
        nc.tensor.matmul(ps_m2b[:], Mt[:], Msb[:], start=True, stop=True)
        M2 = const.tile([128, 128], f32)
        nc.scalar.copy(M2[:], ps_m2b[:])
        ps_m4 = psT.tile([128, 128], f32, tag="pst")
        nc.tensor.matmul(ps_m4[:], M2[:], M2t[:], start=True, stop=True)
        M4t = const.tile([128, 128], f32)
        nc.vector.tensor_copy(M4t[:], ps_m4[:])

        # ---- initial state Z0 = [x0^T | dt^2 I ; v0^T | dt I] ----
        Z0 = chain.tile([128, SD], f32, tag="Z")
        XV = const.tile([NL, 2 * D], f32)
        nc.vector.tensor_copy(XV[:, 0:D], xin[:])
        nc.vector.tensor_copy(XV[:, D:2 * D], vin[:])
        ps_z = psT.tile([128, NL], f32, tag="pst")
        nc.tensor.matmul(ps_z[:], XV[:], ident[0:NL, 0:NL], start=True, stop=True)
        nc.vector.tensor_copy(Z0[:, 0:NL], ps_z[:])
        nc.scalar.mul(Z0[0:D, NL:SD], idA, dt * dt)
        nc.scalar.mul(Z0[D:128, NL:SD], idB, dt)

        # ---- accumulators + scratch ----
        Y = acc.tile([D, PB], f32)            # Y[:, t*NL+n] = x_t^T column n
        G_acc = acc.tile([D, T * D], f32)     # col block t*D = G row t = Mx(T-1-t)
        Gh = acc.tile([D, T * D], bf16)
        Gr = acc.tile([D, T * D], f32)
        Gl = acc.tile([D, T * D], bf16)
        Gst = [
            dram.tile([2, T // 2, D, D], bf16, name=f"gs{th}", tag=f"gs{th}")
            for th in range(2)
        ]
        Xst = [
            dram.tile([2, 2, TPB, NL, D], bf16, name=f"xs{b}", tag=f"xs{b}")
            for b in range(NB)
        ]
        GfatA = acc.tile([128, D * D], bf16)
        GfatB = acc.tile([128, D * D], bf16)
        XfatA = acc.tile([128, NL * D], bf16)
        XfatB = acc.tile([128, NL * D], bf16)
        rr = [nc.sync, nc.scalar]

        def extract(tau, Z):
            """Pull x_tau^T and Mx(tau) out of state Z (off the DVE queue)."""
            if tau % 2 == 0:
                nc.gpsimd.tensor_copy(Y[:, tau * NL:(tau + 1) * NL], Z[0:D, 0:NL])
                nc.gpsimd.tensor_copy(
                    G_acc[:, (T - 1 - tau) * D:(T - tau) * D], Z[0:D, NL:SD]
                )
            else:
                nc.scalar.copy(Y[:, tau * NL:(tau + 1) * NL], Z[0:D, 0:NL])
                nc.scalar.copy(
                    G_acc[:, (T - 1 - tau) * D:(T - tau) * D], Z[0:D, NL:SD]
                )

        def emit_g_half(th):
            """hi/lo-split + scratch store + stack-read of G rows th*T/2 ...."""
            c0, c1 = th * (T // 2) * D, (th + 1) * (T // 2) * D
            nc.vector.tensor_copy(Gh[:, c0:c1], G_acc[:, c0:c1])
            nc.vector.scalar_tensor_tensor(
                Gr[:, c0:c1], Gh[:, c0:c1], -1.0, G_acc[:, c0:c1], op0=mult, op1=add
            )
            nc.vector.tensor_copy(Gl[:, c0:c1], Gr[:, c0:c1])
            for hl, src in ((0, Gh), (1, Gl)):
                rr[hl].dma_start(
                    Gst[th][hl].rearrange("t d i -> d t i"),
                    src[:, c0:c1].rearrange("d (t i) -> d t i", t=T // 2),
                )
            # stack row k = th*32 + q2*16 + hl*8 + tt; G row = G_hl[t], dup in q2
            gsrc = Gst[th][:].rearrange("hl t d i -> (hl t) (d i)")
            m = th
            for rep, fat in ((0, GfatA), (1, GfatB)):
                for q2 in range(2):
                    r0 = rep * K + th * 32 + q2 * 16
                    rr[m % 2].dma_start(fat[r0:r0 + 16, :], gsrc)
                    m += 1

        def emit_block_reshape(b):
            """Transpose Y block b -> natural x rows; traj + hi/lo stack rows."""
            ps_tb = psT.tile([BS, D], f32, tag="pst")
            nc.tensor.transpose(ps_tb[:], Y[:, b * BS:(b + 1) * BS], idA)
            tb = osb.tile([BS, D], f32, tag="tb_sb")
            nc.scalar.copy(tb[:], ps_tb[:])
            for f in range(NF - 1):
                t = f * SE
                if b * BS <= t * NL < (b + 1) * BS:
                    off = t * NL - b * BS
                    nc.sync.dma_start(traj_out[f], tb[off:off + NL, :])
            tbh = osb.tile([BS, D], bf16, tag="tbh")
            nc.vector.tensor_copy(tbh[:], tb[:])
            tbr = osb.tile([BS, D], f32, tag="tbr")
            nc.vector.scalar_tensor_tensor(
                tbr[:], tbh[:], -1.0, tb[:], op0=mult, op1=add
            )
            tbl = osb.tile([BS, D], bf16, tag="tbl")
            nc.vector.tensor_copy(tbl[:], tbr[:])
            m = b
            for xhl, srct in ((0, tbh), (1, tbl)):
                for dup in range(2):
                    rr[m % 2].dma_start(
                        Xst[b][xhl, dup].rearrange("t n j -> (t n) j"), srct[:]
                    )
                    m += 1
            # stack row k = th*32 + q2*16 + dup*8 + ttg; X row = X_q2[t], G row = G_dup[t]
            t0 = b * TPB
            th, off = t0 // (T // 2), t0 % (T // 2)
            for rep, fat in ((0, XfatA), (1, XfatB)):
                for q2 in range(2):
                    for dup in range(2):
                        r0 = rep * K + th * 32 + q2 * 16 + dup * 8 + off
                        rr[m % 2].dma_start(
                            fat[r0:r0 + TPB, :],
                            Xst[b][q2, dup].rearrange("t n j -> t (n j)"),
                        )
                        m += 1

        # ---- the chain: two interleaved double-step sequences ----
        Zs = [None] * (T + 1)
        Zs[0] = Z0
        extract(0, Z0)
        ps1 = psA.tile([128, SD], f32, tag="psz")
        nc.tensor.matmul(ps1[:], Mt[:], Z0[:], start=True, stop=True)
        Z1 = chain.tile([128, SD], f32, tag="Z")
        nc.vector.tensor_copy(Z1[:], ps1[:])
        Zs[1] = Z1
        extract(1, Z1)
        for tau in range(2, T + 1):
            ps = psA.tile([128, SD], f32, tag="psz")
            if tau == 2:
                nc.tensor.matmul(ps[:], M2t[:], Zs[0][:], start=True, stop=True)
            elif tau == 3:
                nc.tensor.matmul(ps[:], M2t[:], Zs[1][:], start=True, stop=True)
            else:
                nc.tensor.matmul(ps[:], M4t[:], Zs[tau - 4][:], start=True, stop=True)
            Z = chain.tile([128, SD], f32, tag="Z")
            if tau % 2 == 0:
                nc.vector.tensor_copy(Z[:], ps[:])
            else:
                nc.scalar.copy(Z[:], ps[:])
            Zs[tau] = Z
            if tau < T:
                extract(tau, Z)
            if (tau + 1) % TPB == 0:
                emit_block_reshape((tau + 1) // TPB - 1)
            if tau == T // 2:
                emit_g_half(1)   # G rows T/2..T-1 came from chain steps 0..T/2-1
        emit_g_half(0)

        # final frame x_T from the last chain state
        ps_xT = psT.tile([NL, D], f32, tag="pst")
        nc.tensor.transpose(ps_xT[:], Zs[T][0:D, 0:NL], idA)
        xT_sb = osb.tile([NL, D], f32, tag="xT_sb")
        nc.scalar.copy(xT_sb[:], ps_xT[:])
        nc.sync.dma_start(traj_out[NF - 1], xT_sb[:])

        # ---- big stage: paired even/odd chunks in concurrent row groups ----
        jac_pair = jac_out.rearrange("n (gq p2) j -> gq n (p2 j)", p2=2)
        m = 0
        for g in range(NG):
            for h in range(NH):
                hs = slice(h * 512, (h + 1) * 512)
                ps = psB.tile([128, 1024], f32)
                nc.tensor.matmul(
                    ps[:, 0:512],
                    GfatA[0:K, g * 256:(g + 1) * 256:2],
                    XfatA[0:K, hs],
                    start=True, stop=True,
                )
                nc.tensor.matmul(
                    ps[:, 512:1024],
                    GfatB[K:2 * K, g * 256 + 1:(g + 1) * 256:2],
                    XfatB[K:2 * K, hs],
                    start=True, stop=True,
                    tile_position=(64, 0),
                )
                ot = osb.tile([128, 1024], f32, tag="ot")
                ov = ot[:].rearrange("q (n p2 j) -> q n p2 j", n=NPH, p2=2)
                iv = ps[:].rearrange("q (p2 n j) -> q n p2 j", p2=2, n=NPH)
                if m % 3 == 2:
                    nc.scalar.copy(ov, iv)
                else:
                    nc.vector.tensor_copy(ov, iv)
                nc.sync.dma_start(
                    jac_pair[g * 128:(g + 1) * 128, h * NPH:(h + 1) * NPH, :],
                    ot[:].rearrange("q (n p2 j) -> q n (p2 j)", n=NPH, p2=2),
                )
                m += 1

    nc.compile()
    return nc


_cache = {}


def _get_nc(T, SE, NL):
    key = (T, SE, NL)
    if key not in _cache:
        _cache[key] = _build_nc(T, SE, NL)
    return _cache[key]


def kernel(x0, v0, force_weight, n_steps, store_every, _trace=False):
    from concourse.bass_utils import run_bass_kernel_spmd

    x0 = np.asarray(x0, dtype=np.float32)
    v0 = np.asarray(v0, dtype=np.float32)
    W = np.asarray(force_weight, dtype=np.float32)
    T = int(n_steps)
    SE = int(store_every)
    n, d = x0.shape
    assert (n, d) == (N, D)
    NL = n // NCORES
    NF = T // SE + 1

    nc = _get_nc(T, SE, NL)
    in_maps = [
        {
            "x0s": np.ascontiguousarray(x0[k * NL:(k + 1) * NL]),
            "v0s": np.ascontiguousarray(v0[k * NL:(k + 1) * NL]),
            "w": np.ascontiguousarray(W),
            "ident": np.eye(128, dtype=np.float32),
        }
        for k in range(NCORES)
    ]
    res = run_bass_kernel_spmd(
        nc, in_maps, core_ids=list(range(NCORES)), trace=_trace
    )
    kernel.last_results = res

    jac = np.concatenate(
        [res.results[k]["jac"].reshape(NL, D, D, D) for k in range(NCORES)], axis=0
    )
    traj = np.concatenate(
        [res.results[k]["traj"] for k in range(NCORES)], axis=1
    )
    assert traj.shape == (NF, n, d) and jac.shape == (n, d, D, D)
    return traj, jac


# revision 19
# speedup vs baseline: 1.0429x; 1.0275x over previous
"""Trainium2 Bass kernel for online forward-mode sensitivity propagation.

Math restructuring: the tangent recurrence for parameter direction p=(i,j)
is linear in (jx, jv) with forcing x_t[n,j] * e_i injected each step.  By
superposition over injection times,

    jac[n,d,i,j] = sum_t Kx(T-1-t)[i,d] * x_t[n,j]

where Kx(tau) is the impulse-response propagator: Kx(0)=dt^2*I, Kv(0)=dt*I,
Kv' = Kv + dt*Kx@(W^T - I), Kx' = Kx + dt*Kv'.  Working with Mx = Kx^T the
propagator recurrence becomes *identical* to the primal state recurrence
(left-multiplied by (W - I)), so one combined 128-row state
Z = [x^T | Mx ; v^T | Mv] advances with a single [128,80] matmul per step:
Z' = M @ Z.  The kernel squares M once and runs TWO interleaved
double-step chains (even/odd), halving the serial dependency length.
The Jacobian then factorizes as, per sample row n,

    jac[n]  (as a [(d,i), j] = [4096, 64] matrix)  =  G @ H_n

with G[(d,i), t] = Mx(T-1-t)[d,i] and H_n[t, j] = x_t[n, j] — a rank-16
contraction instead of propagating 4096 tangent states for 16 steps.

Big-stage implementation notes:
- Operands are split hi/lo in bf16 and stacked along the contraction dim
  (K = 4*T = 64 rows: Gh|Gl|Gh|Gl against Xh|Xh|Xl|Xl), so one bf16 matmul
  computes the full (Gh+Gl)^T (Xh+Xl) product with fp32 PSUM accumulation
  (~1e-7 operand error) — ~4x faster than the PE's fp32 LOW_HIGH mode.
- Outputs are produced in (d,i)-row pairs: even rows via one matmul into
  bank A, odd rows via a second matmul into bank B, run CONCURRENTLY in
  different PE row-groups (tile_position (0,0)/(64,0) with replicated
  operands).  A single strided DVE copy interleaves the pair so the store
  DMA gets 512-byte contiguous runs; the store stream runs at the
  HBM-per-core roofline (~350 GB/s).

Sharding: the 128 sample rows are split 16-per-core across 8 NeuronCores
(rows of x evolve independently); each core computes its own (replicated,
tiny) propagator chain and its 16.8 MB Jacobian shard.
"""

import numpy as np
from contextlib import ExitStack

DT = 0.01
NCORES = 8
N, D = 128, 64  # problem dims (hardcoded per spec)


def _build_nc(T, SE, NL):
    import concourse.bass as bass
    import concourse.tile as tile
    from concourse import bacc, mybir

    f32 = mybir.dt.float32
    bf16 = mybir.dt.bfloat16
    i32 = mybir.dt.int32
    dt = DT
    NF = T // SE + 1            # stored trajectory frames (incl. x0 and x_T)
    PB = T * NL                 # total (t, n) rows for the X reshape
    NB = 2                      # number of transpose blocks
    BS = PB // NB               # Y columns per block
    NH = (NL * D) // 512        # big-matmul free-dim halves
    NPH = 512 // D              # samples per free-dim half
    NG = (D * D) // 256         # 256-row output chunks
    TPB = T // NB               # chain steps per transpose block (= 4)
    K = 4 * T                   # stacked contraction length
    SD = NL + D                 # chain state free width (80)

    nc = bacc.Bacc(
        "TRN2",
        target_bir_lowering=False,
        debug=False,
        enable_asserts=False,
        num_devices=NCORES,
    )

    x0s = nc.dram_tensor("x0s", [NL, D], f32, kind="ExternalInput").ap()
    id_in = nc.dram_tensor("ident", [128, 128], f32, kind="ExternalInput").ap()
    v0s = nc.dram_tensor("v0s", [NL, D], f32, kind="ExternalInput").ap()
    w_in = nc.dram_tensor("w", [D, D], f32, kind="ExternalInput").ap()
    jac_out = nc.dram_tensor("jac", [NL, D * D, D], f32, kind="ExternalOutput").ap()
    traj_out = nc.dram_tensor("traj", [NF, NL, D], f32, kind="ExternalOutput").ap()

    with tile.TileContext(nc) as tc, ExitStack() as ctx:
        const = ctx.enter_context(tc.tile_pool(name="const", bufs=1))
        chain = ctx.enter_context(tc.tile_pool(name="chain", bufs=6))
        acc = ctx.enter_context(tc.tile_pool(name="acc", bufs=1))
        psA = ctx.enter_context(tc.tile_pool(name="psA", bufs=3, space="PSUM"))
        psT = ctx.enter_context(tc.tile_pool(name="psT", bufs=1, space="PSUM"))
        psB = ctx.enter_context(tc.tile_pool(name="psB", bufs=2, space="PSUM"))
        osb = ctx.enter_context(tc.tile_pool(name="osb", bufs=5))
        dram = ctx.enter_context(tc.tile_pool(name="dram", bufs=1, space="DRAM"))

        mult = mybir.AluOpType.mult
        add = mybir.AluOpType.add

        # ---- setup: load inputs; identity built on DVE (gpsimd starts late) ----
        xin = const.tile([NL, D], f32)
        nc.sync.dma_start(xin[:], x0s)
        vin = const.tile([NL, D], f32)
        nc.scalar.dma_start(vin[:], v0s)
        wsb = const.tile([D, D], f32)
        nc.scalar.dma_start(wsb[:], w_in)
        ident = const.tile([128, 128], f32)
        nc.sync.dma_start(ident[:], id_in)
        idA = ident[0:D, 0:D]            # I64 on partitions 0..63
        idB = ident[D:128, D:128]        # I64 on partitions 64..127

        ps_wt = psT.tile([D, D], f32, tag="pst")
        nc.tensor.transpose(ps_wt[:], wsb[:], idA)
        wmi = const.tile([D, D], f32)  # B = W^T - I
        nc.vector.scalar_tensor_tensor(wmi[:], idA, -1.0, ps_wt[:], op0=mult, op1=add)

        # ---- step matrix (transposed): Mt = M^T = [[I+dt^2 B, dt B],[dt I, I]] ----
        Mt = const.tile([128, 128], f32)
        nc.vector.scalar_tensor_tensor(
            Mt[0:D, 0:D], wmi[:], dt * dt, idA, op0=mult, op1=add
        )
        nc.scalar.mul(Mt[0:D, D:128], wmi[:], dt)
        nc.scalar.mul(Mt[D:128, 0:D], idB, dt)
        nc.scalar.copy(Mt[D:128, D:128], idB)

        # M2t = (M^2)^T = M^T M^T  via  matmul(lhsT=M, rhs=Mt)
        ps_m = psT.tile([128, 128], f32, tag="pst")
        nc.tensor.transpose(ps_m[:], Mt[:], ident[:])
        Msb = const.tile([128, 128], f32)
        nc.vector.tensor_copy(Msb[:], ps_m[:])
        ps_m2 = psT.tile([128, 128], f32, tag="pst")
        nc.tensor.matmul(ps_m2[:], Msb[:], Mt[:], start=True, stop=True)
        M2t = const.tile([128, 128], f32)
        nc.vector.tensor_copy(M2t[:], ps_m2[:])
        ps_m2b = psT.tile([128, 128], f32, tag="pst")
        nc.tensor.matmul(ps_m2b[:], Mt[:], Msb[:], start=True, stop=True)
        M2 = const.tile([128, 128], f32)
        nc.scalar.copy(M2[:], ps_m2b[:])
        ps_m4 = psT.tile([128, 128], f32, tag="pst")
        nc.tensor.matmul(ps_m4[:], M2[:], M2t[:], start=True, stop=True)
        M4t = const.tile([128, 128], f32)
        nc.vector.tensor_copy(M4t[:], ps_m4[:])

        # ---- initial state Z0 = [x0^T | dt^2 I ; v0^T | dt I] ----
        Z0 = chain.tile([128, SD], f32, tag="Z")
        XV = const.tile([NL, 2 * D], f32)
        nc.vector.tensor_copy(XV[:, 0:D], xin[:])
        nc.vector.tensor_copy(XV[:, D:2 * D], vin[:])
        ps_z = psT.tile([128, NL], f32, tag="pst")
        nc.tensor.matmul(ps_z[:], XV[:], ident[0:NL, 0:NL], start=True, stop=True)
        nc.vector.tensor_copy(Z0[:, 0:NL], ps_z[:])
        nc.scalar.mul(Z0[0:D, NL:SD], idA, dt * dt)
        nc.scalar.mul(Z0[D:128, NL:SD], idB, dt)

        # ---- accumulators + scratch ----
        Y = acc.tile([D, PB], f32)            # Y[:, t*NL+n] = x_t^T column n
        G_acc = acc.tile([D, T * D], f32)     # col block t*D = G row t = Mx(T-1-t)
        Gh = acc.tile([D, T * D], bf16)
        Gr = acc.tile([D, T * D], f32)
        Gl = acc.tile([D, T * D], bf16)
        Gst = [
            dram.tile([2, T // 2, D, D], bf16, name=f"gs{th}", tag=f"gs{th}")
            for th in range(2)
        ]
        Xst = [
            dram.tile([2, 2, TPB, NL, D], bf16, name=f"xs{b}", tag=f"xs{b}")
            for b in range(NB)
        ]
        GfatA = acc.tile([128, D * D], bf16)
        GfatB = acc.tile([128, D * D], bf16)
        XfatA = acc.tile([128, NL * D], bf16)
        XfatB = acc.tile([128, NL * D], bf16)
        rr = [nc.sync, nc.scalar]

        def extract(tau, Z):
            """Pull x_tau^T and Mx(tau) out of state Z (off the DVE queue)."""
            if tau % 2 == 0:
                nc.gpsimd.tensor_copy(Y[:, tau * NL:(tau + 1) * NL], Z[0:D, 0:NL])
                nc.gpsimd.tensor_copy(
                    G_acc[:, (T - 1 - tau) * D:(T - tau) * D], Z[0:D, NL:SD]
                )
            else:
                nc.scalar.copy(Y[:, tau * NL:(tau + 1) * NL], Z[0:D, 0:NL])
                nc.scalar.copy(
                    G_acc[:, (T - 1 - tau) * D:(T - tau) * D], Z[0:D, NL:SD]
                )

        def emit_g_half(th, post=False):
            """hi/lo-split + scratch store + stack-read of G rows th*T/2 ...."""
            c0, c1 = th * (T // 2) * D, (th + 1) * (T // 2) * D
            nc.vector.tensor_copy(Gh[:, c0:c1], G_acc[:, c0:c1])
            nc.vector.scalar_tensor_tensor(
                Gr[:, c0:c1], Gh[:, c0:c1], -1.0, G_acc[:, c0:c1], op0=mult, op1=add
            )
            nc.vector.tensor_copy(Gl[:, c0:c1], Gr[:, c0:c1])
            for hl, src in ((0, Gh), (1, Gl)):
                rr[hl if post else 0].dma_start(
                    Gst[th][hl].rearrange("t d i -> d t i"),
                    src[:, c0:c1].rearrange("d (t i) -> d t i", t=T // 2),
                )
            # stack row k = th*32 + q2*16 + hl*8 + tt; G row = G_hl[t], dup in q2
            gsrc = Gst[th][:].rearrange("hl t d i -> (hl t) (d i)")
            m = th
            for rep, fat in ((0, GfatA), (1, GfatB)):
                for q2 in range(2):
                    r0 = rep * K + th * 32 + q2 * 16
                    rr[m % 2 if post else 0].dma_start(fat[r0:r0 + 16, :], gsrc)
                    m += 1

        def emit_block_reshape(b, post=False):
            """Transpose Y block b -> natural x rows; traj + hi/lo stack rows."""
            ps_tb = psT.tile([BS, D], f32, tag="pst")
            nc.tensor.transpose(ps_tb[:], Y[:, b * BS:(b + 1) * BS], idA)
            tb = osb.tile([BS, D], f32, tag="tb_sb")
            nc.scalar.copy(tb[:], ps_tb[:])
            for f in range(NF - 1):
                t = f * SE
                if b * BS <= t * NL < (b + 1) * BS:
                    off = t * NL - b * BS
                    nc.sync.dma_start(traj_out[f], tb[off:off + NL, :])
            tbh = osb.tile([BS, D], bf16, tag="tbh")
            nc.vector.tensor_copy(tbh[:], tb[:])
            tbr = osb.tile([BS, D], f32, tag="tbr")
            nc.vector.scalar_tensor_tensor(
                tbr[:], tbh[:], -1.0, tb[:], op0=mult, op1=add
            )
            tbl = osb.tile([BS, D], bf16, tag="tbl")
            nc.vector.tensor_copy(tbl[:], tbr[:])
            m = b
            for xhl, srct in ((0, tbh), (1, tbl)):
                for dup in range(2):
                    rr[m % 2 if post else 0].dma_start(
                        Xst[b][xhl, dup].rearrange("t n j -> (t n) j"), srct[:]
                    )
                    m += 1
            # stack row k = th*32 + q2*16 + dup*8 + tt; X row = X_q2[t], G row = G_dup[t]
            assert TPB == T // 2
            for rep, fat in ((0, XfatA), (1, XfatB)):
                for q2 in range(2):
                    r0 = rep * K + b * 32 + q2 * 16
                    rr[m % 2 if post else 0].dma_start(
                        fat[r0:r0 + 16, :],
                        Xst[b][q2].rearrange("dup t n j -> (dup t) (n j)"),
                    )
                    m += 1

        # ---- the chain: two interleaved double-step sequences ----
        Zs = [None] * (T + 1)
        Zs[0] = Z0
        extract(0, Z0)
        ps1 = psA.tile([128, SD], f32, tag="psz")
        nc.tensor.matmul(ps1[:], Mt[:], Z0[:], start=True, stop=True)
        Z1 = chain.tile([128, SD], f32, tag="Z")
        nc.vector.tensor_copy(Z1[:], ps1[:])
        Zs[1] = Z1
        extract(1, Z1)
        for tau in range(2, T + 1):
            ps = psA.tile([128, SD], f32, tag="psz")
            if tau == 2:
                nc.tensor.matmul(ps[:], M2t[:], Zs[0][:], start=True, stop=True)
            elif tau == 3:
                nc.tensor.matmul(ps[:], M2t[:], Zs[1][:], start=True, stop=True)
            else:
                nc.tensor.matmul(ps[:], M4t[:], Zs[tau - 4][:], start=True, stop=True)
            Z = chain.tile([128, SD], f32, tag="Z")
            if tau % 2 == 0:
                nc.vector.tensor_copy(Z[:], ps[:])
            else:
                nc.scalar.copy(Z[:], ps[:])
            Zs[tau] = Z
            if tau < T:
                extract(tau, Z)
            if (tau + 1) % TPB == 0 and tau < T:
                emit_block_reshape((tau + 1) // TPB - 1)
            if tau == T // 2:
                emit_g_half(1)   # G rows T/2..T-1 came from chain steps 0..T/2-1
        emit_block_reshape(NB - 1, post=True)
        emit_g_half(0, post=True)

        # final frame x_T from the last chain state
        ps_xT = psT.tile([NL, D], f32, tag="pst")
        nc.tensor.transpose(ps_xT[:], Zs[T][0:D, 0:NL], idA)
        xT_sb = osb.tile([NL, D], f32, tag="xT_sb")
        nc.scalar.copy(xT_sb[:], ps_xT[:])
        nc.sync.dma_start(traj_out[NF - 1], xT_sb[:])

        # ---- big stage: paired even/odd chunks in concurrent row groups ----
        jac_pair = jac_out.rearrange("n (gq p2) j -> gq n (p2 j)", p2=2)
        m = 0
        for g in range(NG):
            for h in range(NH):
                hs = slice(h * 512, (h + 1) * 512)
                ps = psB.tile([128, 1024], f32)
                nc.tensor.matmul(
                    ps[:, 0:512],
                    GfatA[0:K, g * 256:(g + 1) * 256:2],
                    XfatA[0:K, hs],
                    start=True, stop=True,
                )
                nc.tensor.matmul(
                    ps[:, 512:1024],
                    GfatB[K:2 * K, g * 256 + 1:(g + 1) * 256:2],
                    XfatB[K:2 * K, hs],
                    start=True, stop=True,
                    tile_position=(64, 0),
                )
                ot = osb.tile([128, 1024], f32, tag="ot")
                ov = ot[:].rearrange("q (n p2 j) -> q n p2 j", n=NPH, p2=2)
                iv = ps[:].rearrange("q (p2 n j) -> q n p2 j", p2=2, n=NPH)
                if m % 3 == 2:
                    nc.scalar.copy(ov, iv)
                else:
                    nc.vector.tensor_copy(ov, iv)
                nc.sync.dma_start(
                    jac_pair[g * 128:(g + 1) * 128, h * NPH:(h + 1) * NPH, :],
                    ot[:].rearrange("q (n p2 j) -> q n (p2 j)", n=NPH, p2=2),
                )
                m += 1

    nc.compile()
    return nc


_cache = {}


def _get_nc(T, SE, NL):
    key = (T, SE, NL)
    if key not in _cache:
        _cache[key] = _build_nc(T, SE, NL)
    return _cache[key]


def kernel(x0, v0, force_weight, n_steps, store_every, _trace=False):
    from concourse.bass_utils import run_bass_kernel_spmd

    x0 = np.asarray(x0, dtype=np.float32)
    v0 = np.asarray(v0, dtype=np.float32)
    W = np.asarray(force_weight, dtype=np.float32)
    T = int(n_steps)
    SE = int(store_every)
    n, d = x0.shape
    assert (n, d) == (N, D)
    NL = n // NCORES
    NF = T // SE + 1

    nc = _get_nc(T, SE, NL)
    in_maps = [
        {
            "x0s": np.ascontiguousarray(x0[k * NL:(k + 1) * NL]),
            "v0s": np.ascontiguousarray(v0[k * NL:(k + 1) * NL]),
            "w": np.ascontiguousarray(W),
            "ident": np.eye(128, dtype=np.float32),
        }
        for k in range(NCORES)
    ]
    res = run_bass_kernel_spmd(
        nc, in_maps, core_ids=list(range(NCORES)), trace=_trace
    )
    kernel.last_results = res

    jac = np.concatenate(
        [res.results[k]["jac"].reshape(NL, D, D, D) for k in range(NCORES)], axis=0
    )
    traj = np.concatenate(
        [res.results[k]["traj"] for k in range(NCORES)], axis=1
    )
    assert traj.shape == (NF, n, d) and jac.shape == (n, d, D, D)
    return traj, jac


# revision 20
# speedup vs baseline: 1.1016x; 1.0563x over previous
"""Trainium2 Bass kernel for online forward-mode sensitivity propagation.

Math restructuring: the tangent recurrence for parameter direction p=(i,j)
is linear in (jx, jv) with forcing x_t[n,j] * e_i injected each step.  By
superposition over injection times,

    jac[n,d,i,j] = sum_t Kx(T-1-t)[i,d] * x_t[n,j]

where Kx(tau) is the impulse-response propagator: Kx(0)=dt^2*I, Kv(0)=dt*I,
Kv' = Kv + dt*Kx@(W^T - I), Kx' = Kx + dt*Kv'.  Working with Mx = Kx^T the
propagator recurrence becomes *identical* to the primal state recurrence
(left-multiplied by (W - I)), so one combined 128-row state
Z = [x^T | Mx ; v^T | Mv] advances with a single [128,80] matmul per step:
Z' = M @ Z.  The kernel squares M once and runs TWO interleaved
double-step chains (even/odd), halving the serial dependency length.
The Jacobian then factorizes as, per sample row n,

    jac[n]  (as a [(d,i), j] = [4096, 64] matrix)  =  G @ H_n

with G[(d,i), t] = Mx(T-1-t)[d,i] and H_n[t, j] = x_t[n, j] — a rank-16
contraction instead of propagating 4096 tangent states for 16 steps.

Big-stage implementation notes:
- Operands are split hi/lo in bf16 and stacked along the contraction dim
  (K = 4*T = 64 rows: Gh|Gl|Gh|Gl against Xh|Xh|Xl|Xl), so one bf16 matmul
  computes the full (Gh+Gl)^T (Xh+Xl) product with fp32 PSUM accumulation
  (~1e-7 operand error) — ~4x faster than the PE's fp32 LOW_HIGH mode.
- Outputs are produced in (d,i)-row pairs: even rows via one matmul into
  bank A, odd rows via a second matmul into bank B, run CONCURRENTLY in
  different PE row-groups (tile_position (0,0)/(64,0) with replicated
  operands).  A single strided DVE copy interleaves the pair so the store
  DMA gets 512-byte contiguous runs; the store stream runs at the
  HBM-per-core roofline (~350 GB/s).

Sharding: the 128 sample rows are split 16-per-core across 8 NeuronCores
(rows of x evolve independently); each core computes its own (replicated,
tiny) propagator chain and its 16.8 MB Jacobian shard.
"""

import numpy as np
from contextlib import ExitStack

DT = 0.01
NCORES = 8
N, D = 128, 64  # problem dims (hardcoded per spec)


def _build_nc(T, SE, NL):
    import concourse.bass as bass
    import concourse.tile as tile
    from concourse import bacc, mybir

    f32 = mybir.dt.float32
    bf16 = mybir.dt.bfloat16
    i32 = mybir.dt.int32
    dt = DT
    NF = T // SE + 1            # stored trajectory frames (incl. x0 and x_T)
    PB = T * NL                 # total (t, n) rows for the X reshape
    NB = 2                      # number of transpose blocks
    BS = PB // NB               # Y columns per block
    NH = (NL * D) // 512        # big-matmul free-dim halves
    NPH = 512 // D              # samples per free-dim half
    NG = (D * D) // 256         # 256-row output chunks
    TPB = T // NB               # chain steps per transpose block (= 4)
    K = 4 * T                   # stacked contraction length
    SD = NL + D                 # chain state free width (80)

    nc = bacc.Bacc(
        "TRN2",
        target_bir_lowering=False,
        debug=False,
        enable_asserts=False,
        num_devices=NCORES,
    )

    x0s = nc.dram_tensor("x0s", [NL, D], f32, kind="ExternalInput").ap()
    id_in = nc.dram_tensor("ident", [128, 128], f32, kind="ExternalInput").ap()
    v0s = nc.dram_tensor("v0s", [NL, D], f32, kind="ExternalInput").ap()
    w_in = nc.dram_tensor("w", [D, D], f32, kind="ExternalInput").ap()
    jac_out = nc.dram_tensor("jac", [NL, D * D, D], f32, kind="ExternalOutput").ap()
    traj_out = nc.dram_tensor("traj", [NF, NL, D], f32, kind="ExternalOutput").ap()

    with tile.TileContext(nc) as tc, ExitStack() as ctx:
        const = ctx.enter_context(tc.tile_pool(name="const", bufs=1))
        chain = ctx.enter_context(tc.tile_pool(name="chain", bufs=6))
        acc = ctx.enter_context(tc.tile_pool(name="acc", bufs=1))
        psA = ctx.enter_context(tc.tile_pool(name="psA", bufs=3, space="PSUM"))
        psT = ctx.enter_context(tc.tile_pool(name="psT", bufs=1, space="PSUM"))
        psB = ctx.enter_context(tc.tile_pool(name="psB", bufs=2, space="PSUM"))
        osb = ctx.enter_context(tc.tile_pool(name="osb", bufs=5))
        dram = ctx.enter_context(tc.tile_pool(name="dram", bufs=1, space="DRAM"))

        mult = mybir.AluOpType.mult
        add = mybir.AluOpType.add

        # ---- setup: load inputs; identity built on DVE (gpsimd starts late) ----
        xin = const.tile([NL, D], f32)
        nc.sync.dma_start(xin[:], x0s)
        vin = const.tile([NL, D], f32)
        nc.scalar.dma_start(vin[:], v0s)
        wsb = const.tile([D, D], f32)
        nc.scalar.dma_start(wsb[:], w_in)
        ident = const.tile([128, 128], f32)
        nc.sync.dma_start(ident[:], id_in)
        idA = ident[0:D, 0:D]            # I64 on partitions 0..63
        idB = ident[D:128, D:128]        # I64 on partitions 64..127

        ps_wt = psT.tile([D, D], f32, tag="pst")
        nc.tensor.transpose(ps_wt[:], wsb[:], idA)
        wmi = const.tile([D, D], f32)  # B = W^T - I
        nc.vector.scalar_tensor_tensor(wmi[:], idA, -1.0, ps_wt[:], op0=mult, op1=add)

        # ---- step matrix (transposed): Mt = M^T = [[I+dt^2 B, dt B],[dt I, I]] ----
        Mt = const.tile([128, 128], f32)
        nc.vector.scalar_tensor_tensor(
            Mt[0:D, 0:D], wmi[:], dt * dt, idA, op0=mult, op1=add
        )
        nc.scalar.mul(Mt[0:D, D:128], wmi[:], dt)
        nc.scalar.mul(Mt[D:128, 0:D], idB, dt)
        nc.scalar.copy(Mt[D:128, D:128], idB)

        # M2t = (M^2)^T = M^T M^T  via  matmul(lhsT=M, rhs=Mt)
        ps_m = psT.tile([128, 128], f32, tag="pst")
        nc.tensor.transpose(ps_m[:], Mt[:], ident[:])
        Msb = const.tile([128, 128], f32)
        nc.vector.tensor_copy(Msb[:], ps_m[:])
        ps_m2 = psT.tile([128, 128], f32, tag="pst")
        nc.tensor.matmul(ps_m2[:], Msb[:], Mt[:], start=True, stop=True)
        M2t = const.tile([128, 128], f32)
        nc.vector.tensor_copy(M2t[:], ps_m2[:])
        ps_m2b = psT.tile([128, 128], f32, tag="pst")
        nc.tensor.matmul(ps_m2b[:], Mt[:], Msb[:], start=True, stop=True)
        M2 = const.tile([128, 128], f32)
        nc.scalar.copy(M2[:], ps_m2b[:])
        ps_m4 = psT.tile([128, 128], f32, tag="pst")
        nc.tensor.matmul(ps_m4[:], M2[:], M2t[:], start=True, stop=True)
        M4t = const.tile([128, 128], f32)
        nc.vector.tensor_copy(M4t[:], ps_m4[:])

        # ---- initial state Z0 = [x0^T | dt^2 I ; v0^T | dt I] ----
        Z0 = chain.tile([128, SD], f32, tag="Z")
        XV = const.tile([NL, 2 * D], f32)
        nc.vector.tensor_copy(XV[:, 0:D], xin[:])
        nc.vector.tensor_copy(XV[:, D:2 * D], vin[:])
        ps_z = psT.tile([128, NL], f32, tag="pst")
        nc.tensor.matmul(ps_z[:], XV[:], ident[0:NL, 0:NL], start=True, stop=True)
        nc.vector.tensor_copy(Z0[:, 0:NL], ps_z[:])
        nc.scalar.mul(Z0[0:D, NL:SD], idA, dt * dt)
        nc.scalar.mul(Z0[D:128, NL:SD], idB, dt)

        # ---- accumulators + scratch ----
        Y = acc.tile([D, PB], f32)            # Y[:, t*NL+n] = x_t^T column n
        G_acc = acc.tile([D, T * D], f32)     # col block t*D = G row t = Mx(T-1-t)
        Gh = acc.tile([D, T * D], bf16)
        Gr = acc.tile([D, T * D], f32)
        Gl = acc.tile([D, T * D], bf16)
        Gst = [
            dram.tile([2, T // 2, D, D], bf16, name=f"gs{th}", tag=f"gs{th}")
            for th in range(2)
        ]
        Xst = [
            dram.tile([2, 2, TPB, NL, D], bf16, name=f"xs{b}", tag=f"xs{b}")
            for b in range(NB)
        ]
        GfatA = acc.tile([128, D * D], bf16)
        GfatB = acc.tile([128, D * D], bf16)
        XfatA = acc.tile([128, NL * D], bf16)
        XfatB = acc.tile([128, NL * D], bf16)
        rr = [nc.sync, nc.scalar]

        def extract(tau, Z):
            """Pull x_tau^T and Mx(tau) out of state Z (off the DVE queue)."""
            if tau % 2 == 0:
                nc.gpsimd.tensor_copy(Y[:, tau * NL:(tau + 1) * NL], Z[0:D, 0:NL])
                nc.gpsimd.tensor_copy(
                    G_acc[:, (T - 1 - tau) * D:(T - tau) * D], Z[0:D, NL:SD]
                )
            else:
                nc.scalar.copy(Y[:, tau * NL:(tau + 1) * NL], Z[0:D, 0:NL])
                nc.scalar.copy(
                    G_acc[:, (T - 1 - tau) * D:(T - tau) * D], Z[0:D, NL:SD]
                )

        def emit_g_half(th, post=False):
            """hi/lo-split + scratch store + stack-read of G rows th*T/2 ...."""
            c0, c1 = th * (T // 2) * D, (th + 1) * (T // 2) * D
            nc.vector.tensor_copy(Gh[:, c0:c1], G_acc[:, c0:c1])
            nc.vector.scalar_tensor_tensor(
                Gr[:, c0:c1], Gh[:, c0:c1], -1.0, G_acc[:, c0:c1], op0=mult, op1=add
            )
            nc.vector.tensor_copy(Gl[:, c0:c1], Gr[:, c0:c1])
            for hl, src in ((0, Gh), (1, Gl)):
                rr[hl if post else 0].dma_start(
                    Gst[th][hl].rearrange("t d i -> d t i"),
                    src[:, c0:c1].rearrange("d (t i) -> d t i", t=T // 2),
                )
            # stack row k = th*32 + q2*16 + hl*8 + tt; G row = G_hl[t], dup in q2
            gsrc = Gst[th][:].rearrange("hl t d i -> (hl t) (d i)")
            m = th
            for rep, fat in ((0, GfatA), (1, GfatB)):
                for q2 in range(2):
                    r0 = rep * K + th * 32 + q2 * 16
                    rr[m % 2 if post else 0].dma_start(fat[r0:r0 + 16, :], gsrc)
                    m += 1

        def emit_block_reshape(b, post=False):
            """Transpose Y block b -> natural x rows; traj + hi/lo stack rows."""
            ps_tb = psT.tile([BS, D], f32, tag="pst")
            nc.tensor.transpose(ps_tb[:], Y[:, b * BS:(b + 1) * BS], idA)
            tb = osb.tile([BS, D], f32, tag="tb_sb")
            nc.scalar.copy(tb[:], ps_tb[:])
            for f in range(NF - 1):
                t = f * SE
                if b * BS <= t * NL < (b + 1) * BS:
                    off = t * NL - b * BS
                    nc.sync.dma_start(traj_out[f], tb[off:off + NL, :])
            tbh = osb.tile([BS, D], bf16, tag="tbh")
            nc.vector.tensor_copy(tbh[:], tb[:])
            tbr = osb.tile([BS, D], f32, tag="tbr")
            nc.vector.scalar_tensor_tensor(
                tbr[:], tbh[:], -1.0, tb[:], op0=mult, op1=add
            )
            tbl = osb.tile([BS, D], bf16, tag="tbl")
            nc.vector.tensor_copy(tbl[:], tbr[:])
            m = b
            for xhl, srct in ((0, tbh), (1, tbl)):
                for dup in range(2):
                    rr[m % 2 if post else 0].dma_start(
                        Xst[b][xhl, dup].rearrange("t n j -> (t n) j"), srct[:]
                    )
                    m += 1
            # stack row k = th*32 + q2*16 + dup*8 + tt; X row = X_q2[t], G row = G_dup[t]
            assert TPB == T // 2
            for rep, fat in ((0, XfatA), (1, XfatB)):
                for q2 in range(2):
                    r0 = rep * K + b * 32 + q2 * 16
                    rr[m % 2 if post else 0].dma_start(
                        fat[r0:r0 + 16, :],
                        Xst[b][q2].rearrange("dup t n j -> (dup t) (n j)"),
                    )
                    m += 1

        # ---- the chain: two interleaved double-step sequences ----
        Zs = [None] * (T + 1)
        Zs[0] = Z0
        extract(0, Z0)
        ps1 = psA.tile([128, SD], f32, tag="psz")
        nc.tensor.matmul(ps1[:], Mt[:], Z0[:], start=True, stop=True)
        Z1 = chain.tile([128, SD], f32, tag="Z")
        nc.vector.tensor_copy(Z1[:], ps1[:])
        Zs[1] = Z1
        extract(1, Z1)
        for tau in range(2, T + 1):
            ps = psA.tile([128, SD], f32, tag="psz")
            if tau == 2:
                nc.tensor.matmul(ps[:], M2t[:], Zs[0][:], start=True, stop=True)
            elif tau == 3:
                nc.tensor.matmul(ps[:], M2t[:], Zs[1][:], start=True, stop=True)
            else:
                nc.tensor.matmul(ps[:], M4t[:], Zs[tau - 4][:], start=True, stop=True)
            Z = chain.tile([128, SD], f32, tag="Z")
            if tau % 2 == 0:
                nc.vector.tensor_copy(Z[:], ps[:])
            else:
                nc.scalar.copy(Z[:], ps[:])
            Zs[tau] = Z
            if tau < T:
                extract(tau, Z)
            if (tau + 1) % TPB == 0 and (tau + 1) // TPB < NB:
                emit_block_reshape((tau + 1) // TPB - 1)
            if tau == T // 2:
                emit_g_half(1)   # G rows T/2..T-1 came from chain steps 0..T/2-1
        emit_block_reshape(NB - 1, post=True)
        emit_g_half(0, post=True)

        # final frame x_T from the last chain state
        ps_xT = psT.tile([NL, D], f32, tag="pst")
        nc.tensor.transpose(ps_xT[:], Zs[T][0:D, 0:NL], idA)
        xT_sb = osb.tile([NL, D], f32, tag="xT_sb")
        nc.scalar.copy(xT_sb[:], ps_xT[:])
        nc.sync.dma_start(traj_out[NF - 1], xT_sb[:])

        # ---- big stage: paired even/odd chunks in concurrent row groups ----
        jac_pair = jac_out.rearrange("n (gq p2) j -> gq n (p2 j)", p2=2)
        m = 0
        for g in range(NG):
            for h in range(NH):
                hs = slice(h * 512, (h + 1) * 512)
                ps = psB.tile([128, 1024], f32)
                nc.tensor.matmul(
                    ps[:, 0:512],
                    GfatA[0:K, g * 256:(g + 1) * 256:2],
                    XfatA[0:K, hs],
                    start=True, stop=True,
                )
                nc.tensor.matmul(
                    ps[:, 512:1024],
                    GfatB[K:2 * K, g * 256 + 1:(g + 1) * 256:2],
                    XfatB[K:2 * K, hs],
                    start=True, stop=True,
                    tile_position=(64, 0),
                )
                ot = osb.tile([128, 1024], f32, tag="ot")
                ov = ot[:].rearrange("q (n p2 j) -> q n p2 j", n=NPH, p2=2)
                iv = ps[:].rearrange("q (p2 n j) -> q n p2 j", p2=2, n=NPH)
                if m % 3 == 2:
                    nc.scalar.copy(ov, iv)
                else:
                    nc.vector.tensor_copy(ov, iv)
                nc.sync.dma_start(
                    jac_pair[g * 128:(g + 1) * 128, h * NPH:(h + 1) * NPH, :],
                    ot[:].rearrange("q (n p2 j) -> q n (p2 j)", n=NPH, p2=2),
                )
                m += 1

    nc.compile()
    return nc


_cache = {}


def _get_nc(T, SE, NL):
    key = (T, SE, NL)
    if key not in _cache:
        _cache[key] = _build_nc(T, SE, NL)
    return _cache[key]


def kernel(x0, v0, force_weight, n_steps, store_every, _trace=False):
    from concourse.bass_utils import run_bass_kernel_spmd

    x0 = np.asarray(x0, dtype=np.float32)
    v0 = np.asarray(v0, dtype=np.float32)
    W = np.asarray(force_weight, dtype=np.float32)
    T = int(n_steps)
    SE = int(store_every)
    n, d = x0.shape
    assert (n, d) == (N, D)
    NL = n // NCORES
    NF = T // SE + 1

    nc = _get_nc(T, SE, NL)
    in_maps = [
        {
            "x0s": np.ascontiguousarray(x0[k * NL:(k + 1) * NL]),
            "v0s": np.ascontiguousarray(v0[k * NL:(k + 1) * NL]),
            "w": np.ascontiguousarray(W),
            "ident": np.eye(128, dtype=np.float32),
        }
        for k in range(NCORES)
    ]
    res = run_bass_kernel_spmd(
        nc, in_maps, core_ids=list(range(NCORES)), trace=_trace
    )
    kernel.last_results = res

    jac = np.concatenate(
        [res.results[k]["jac"].reshape(NL, D, D, D) for k in range(NCORES)], axis=0
    )
    traj = np.concatenate(
        [res.results[k]["traj"] for k in range(NCORES)], axis=1
    )
    assert traj.shape == (NF, n, d) and jac.shape == (n, d, D, D)
    return traj, jac


# revision 22
# speedup vs baseline: 1.1574x; 1.0506x over previous
"""Trainium2 Bass kernel for online forward-mode sensitivity propagation.

Math restructuring: the tangent recurrence for parameter direction p=(i,j)
is linear in (jx, jv) with forcing x_t[n,j] * e_i injected each step.  By
superposition over injection times,

    jac[n,d,i,j] = sum_t Kx(T-1-t)[i,d] * x_t[n,j]

where Kx(tau) is the impulse-response propagator: Kx(0)=dt^2*I, Kv(0)=dt*I,
Kv' = Kv + dt*Kx@(W^T - I), Kx' = Kx + dt*Kv'.  Working with Mx = Kx^T the
propagator recurrence becomes *identical* to the primal state recurrence
(left-multiplied by (W - I)), so one combined 128-row state
Z = [x^T | Mx ; v^T | Mv] advances with a single [128,80] matmul per step:
Z' = M @ Z.  The kernel squares M once and runs TWO interleaved
double-step chains (even/odd), halving the serial dependency length.
The Jacobian then factorizes as, per sample row n,

    jac[n]  (as a [(d,i), j] = [4096, 64] matrix)  =  G @ H_n

with G[(d,i), t] = Mx(T-1-t)[d,i] and H_n[t, j] = x_t[n, j] — a rank-16
contraction instead of propagating 4096 tangent states for 16 steps.

Big-stage implementation notes:
- Operands are split hi/lo in bf16 and stacked along the contraction dim
  (K = 4*T = 64 rows: Gh|Gl|Gh|Gl against Xh|Xh|Xl|Xl), so one bf16 matmul
  computes the full (Gh+Gl)^T (Xh+Xl) product with fp32 PSUM accumulation
  (~1e-7 operand error) — ~4x faster than the PE's fp32 LOW_HIGH mode.
- Outputs are produced in (d,i)-row pairs: even rows via one matmul into
  bank A, odd rows via a second matmul into bank B, run CONCURRENTLY in
  different PE row-groups (tile_position (0,0)/(64,0) with replicated
  operands).  A single strided DVE copy interleaves the pair so the store
  DMA gets 512-byte contiguous runs; the store stream runs at the
  HBM-per-core roofline (~350 GB/s).

Sharding: the 128 sample rows are split 16-per-core across 8 NeuronCores
(rows of x evolve independently); each core computes its own (replicated,
tiny) propagator chain and its 16.8 MB Jacobian shard.
"""

import numpy as np
from contextlib import ExitStack

DT = 0.01
NCORES = 8
N, D = 128, 64  # problem dims (hardcoded per spec)


def _build_nc(T, SE, NL):
    import concourse.bass as bass
    import concourse.tile as tile
    from concourse import bacc, mybir

    f32 = mybir.dt.float32
    bf16 = mybir.dt.bfloat16
    i32 = mybir.dt.int32
    dt = DT
    NF = T // SE + 1            # stored trajectory frames (incl. x0 and x_T)
    PB = T * NL                 # total (t, n) rows for the X reshape
    NB = 2                      # number of transpose blocks
    BS = PB // NB               # Y columns per block
    NH = (NL * D) // 512        # big-matmul free-dim halves
    NPH = 512 // D              # samples per free-dim half
    NG = (D * D) // 256         # 256-row output chunks
    TPB = T // NB               # chain steps per transpose block (= 4)
    K = 4 * T                   # stacked contraction length
    SD = NL + D                 # chain state free width (80)

    nc = bacc.Bacc(
        "TRN2",
        target_bir_lowering=False,
        debug=False,
        enable_asserts=False,
        num_devices=NCORES,
    )

    x0t_in = nc.dram_tensor("x0t", [D, NL], f32, kind="ExternalInput").ap()
    id_in = nc.dram_tensor("ident", [128, 128], f32, kind="ExternalInput").ap()
    v0t_in = nc.dram_tensor("v0t", [D, NL], f32, kind="ExternalInput").ap()
    wt_in = nc.dram_tensor("wt", [D, D], f32, kind="ExternalInput").ap()
    jac_out = nc.dram_tensor("jac", [NL, D * D, D], f32, kind="ExternalOutput").ap()
    traj_out = nc.dram_tensor("traj", [NF, NL, D], f32, kind="ExternalOutput").ap()

    with tile.TileContext(nc) as tc, ExitStack() as ctx:
        const = ctx.enter_context(tc.tile_pool(name="const", bufs=1))
        chain = ctx.enter_context(tc.tile_pool(name="chain", bufs=6))
        acc = ctx.enter_context(tc.tile_pool(name="acc", bufs=1))
        psA = ctx.enter_context(tc.tile_pool(name="psA", bufs=3, space="PSUM"))
        psT = ctx.enter_context(tc.tile_pool(name="psT", bufs=1, space="PSUM"))
        psB = ctx.enter_context(tc.tile_pool(name="psB", bufs=2, space="PSUM"))
        osb = ctx.enter_context(tc.tile_pool(name="osb", bufs=5))
        dram = ctx.enter_context(tc.tile_pool(name="dram", bufs=1, space="DRAM"))

        mult = mybir.AluOpType.mult
        add = mybir.AluOpType.add

        # ---- setup: load inputs; identity built on DVE (gpsimd starts late) ----
        wtsb = const.tile([D, D], f32)
        nc.scalar.dma_start(wtsb[:], wt_in)
        ident = const.tile([128, 128], f32)
        nc.sync.dma_start(ident[:], id_in)
        idA = ident[0:D, 0:D]            # I64 on partitions 0..63
        idB = ident[D:128, D:128]        # I64 on partitions 64..127

        wmi = const.tile([D, D], f32)  # B = W^T - I
        nc.vector.scalar_tensor_tensor(wmi[:], idA, -1.0, wtsb[:], op0=mult, op1=add)

        # ---- step matrix (transposed): Mt = M^T = [[I+dt^2 B, dt B],[dt I, I]] ----
        Mt = const.tile([128, 128], f32)
        nc.vector.scalar_tensor_tensor(
            Mt[0:D, 0:D], wmi[:], dt * dt, idA, op0=mult, op1=add
        )
        nc.scalar.mul(Mt[0:D, D:128], wmi[:], dt)
        nc.scalar.mul(Mt[D:128, 0:D], idB, dt)
        nc.scalar.copy(Mt[D:128, D:128], idB)

        # M2t = (M^2)^T = M^T M^T  via  matmul(lhsT=M, rhs=Mt)
        ps_m = psT.tile([128, 128], f32, tag="pst")
        nc.tensor.transpose(ps_m[:], Mt[:], ident[:])
        Msb = const.tile([128, 128], f32)
        nc.vector.tensor_copy(Msb[:], ps_m[:])
        ps_m2 = psT.tile([128, 128], f32, tag="pst")
        nc.tensor.matmul(ps_m2[:], Msb[:], Mt[:], start=True, stop=True)
        M2t = const.tile([128, 128], f32)
        nc.vector.tensor_copy(M2t[:], ps_m2[:])
        ps_m2b = psT.tile([128, 128], f32, tag="pst")
        nc.tensor.matmul(ps_m2b[:], Mt[:], Msb[:], start=True, stop=True)
        M2 = const.tile([128, 128], f32)
        nc.scalar.copy(M2[:], ps_m2b[:])
        ps_m4 = psT.tile([128, 128], f32, tag="pst")
        nc.tensor.matmul(ps_m4[:], M2[:], M2t[:], start=True, stop=True)
        M4t = const.tile([128, 128], f32)
        nc.vector.tensor_copy(M4t[:], ps_m4[:])

        # ---- initial state Z0 = [x0^T | dt^2 I ; v0^T | dt I] ----
        Z0 = chain.tile([128, SD], f32, tag="Z")
        nc.sync.dma_start(Z0[0:D, 0:NL], x0t_in)
        nc.scalar.dma_start(Z0[D:128, 0:NL], v0t_in)
        nc.scalar.mul(Z0[0:D, NL:SD], idA, dt * dt)
        nc.scalar.mul(Z0[D:128, NL:SD], idB, dt)

        # ---- accumulators + scratch ----
        Y = acc.tile([D, PB], f32)            # Y[:, t*NL+n] = x_t^T column n
        G_acc = acc.tile([D, T * D], f32)     # col block t*D = G row t = Mx(T-1-t)
        Ghl = acc.tile([D, 2 * T * D], bf16)
        Gr = acc.tile([D, T * D], f32)
        Gst = [
            dram.tile([2, T // 2, D, D], bf16, name=f"gs{th}", tag=f"gs{th}")
            for th in range(2)
        ]
        Xst = [
            dram.tile([2, 2, TPB, NL, D], bf16, name=f"xs{b}", tag=f"xs{b}")
            for b in range(NB)
        ]
        GfatA = acc.tile([128, D * D], bf16)
        GfatB = acc.tile([128, D * D], bf16)
        XfatA = acc.tile([128, NL * D], bf16)
        XfatB = acc.tile([128, NL * D], bf16)
        rr = [nc.sync, nc.scalar]

        def extract(tau, Z):
            """Pull x_tau^T and Mx(tau) out of state Z (off the DVE queue)."""
            if tau % 2 == 0:
                nc.gpsimd.tensor_copy(Y[:, tau * NL:(tau + 1) * NL], Z[0:D, 0:NL])
                nc.gpsimd.tensor_copy(
                    G_acc[:, (T - 1 - tau) * D:(T - tau) * D], Z[0:D, NL:SD]
                )
            else:
                nc.scalar.copy(Y[:, tau * NL:(tau + 1) * NL], Z[0:D, 0:NL])
                nc.scalar.copy(
                    G_acc[:, (T - 1 - tau) * D:(T - tau) * D], Z[0:D, NL:SD]
                )

        def emit_g_half(th, post=False):
            """hi/lo-split + scratch store + stack-read of G rows th*T/2 ...."""
            c0, c1 = th * (T // 2) * D, (th + 1) * (T // 2) * D
            TD = T * D
            nc.vector.tensor_copy(Ghl[:, c0:c1], G_acc[:, c0:c1])
            nc.vector.scalar_tensor_tensor(
                Gr[:, c0:c1], Ghl[:, c0:c1], -1.0, G_acc[:, c0:c1], op0=mult, op1=add
            )
            nc.vector.tensor_copy(Ghl[:, TD + c0:TD + c1], Gr[:, c0:c1])
            for hl in range(2):
                rr[hl if post else 0].dma_start(
                    Gst[th][hl].rearrange("t d i -> d t i"),
                    Ghl[:, hl * TD + c0:hl * TD + c1].rearrange(
                        "d (t i) -> d t i", t=T // 2
                    ),
                )
            # stack row k = th*32 + q2*16 + hl*8 + tt; G row = G_hl[t], dup in q2
            gsrc = Gst[th][:].rearrange("hl t d i -> (hl t) (d i)")
            m = th
            for rep, fat in ((0, GfatA), (1, GfatB)):
                for q2 in range(2):
                    r0 = rep * K + th * 32 + q2 * 16
                    rr[m % 2 if post else 0].dma_start(fat[r0:r0 + 16, :], gsrc)
                    m += 1

        def emit_block_reshape(b, post=False):
            """Transpose Y block b -> natural x rows; traj + hi/lo stack rows."""
            ps_tb = psT.tile([BS, D], f32, tag="pst")
            nc.tensor.transpose(ps_tb[:], Y[:, b * BS:(b + 1) * BS], idA)
            tb = osb.tile([BS, D], f32, tag="tb_sb")
            nc.scalar.copy(tb[:], ps_tb[:])
            for f in range(NF - 1):
                t = f * SE
                if b * BS <= t * NL < (b + 1) * BS:
                    off = t * NL - b * BS
                    nc.sync.dma_start(traj_out[f], tb[off:off + NL, :])
            tbx = osb.tile([BS, 2 * D], bf16, tag="tbx")
            nc.vector.tensor_copy(tbx[:, 0:D], tb[:])
            tbr = osb.tile([BS, D], f32, tag="tbr")
            nc.vector.scalar_tensor_tensor(
                tbr[:], tbx[:, 0:D], -1.0, tb[:], op0=mult, op1=add
            )
            nc.vector.tensor_copy(tbx[:, D:2 * D], tbr[:])
            m = b
            tbx_v = tbx[:].rearrange("p (hl j) -> p hl j", hl=2)
            for dup in range(2):
                rr[m % 2 if post else 0].dma_start(
                    Xst[b][:, dup].rearrange("xhl t n j -> (t n) xhl j"), tbx_v
                )
                m += 1
            # stack row k = b*32 + q2*16 + dup*8 + tt; X row = X_q2[t], G row = G_dup[t]
            assert TPB == T // 2
            for rep, fat in ((0, XfatA), (1, XfatB)):
                r0 = rep * K + b * 32
                rr[m % 2 if post else 0].dma_start(
                    fat[r0:r0 + 32, :],
                    Xst[b][:].rearrange("xhl dup t n j -> (xhl dup t) (n j)"),
                )
                m += 1

        # ---- the chain: two interleaved double-step sequences ----
        Zs = [None] * (T + 1)
        Zs[0] = Z0
        extract(0, Z0)
        ps1 = psA.tile([128, SD], f32, tag="psz")
        nc.tensor.matmul(ps1[:], Mt[:], Z0[:], start=True, stop=True)
        Z1 = chain.tile([128, SD], f32, tag="Z")
        nc.vector.tensor_copy(Z1[:], ps1[:])
        Zs[1] = Z1
        extract(1, Z1)
        for tau in range(2, T + 1):
            ps = psA.tile([128, SD], f32, tag="psz")
            if tau == 2:
                nc.tensor.matmul(ps[:], M2t[:], Zs[0][:], start=True, stop=True)
            elif tau == 3:
                nc.tensor.matmul(ps[:], M2t[:], Zs[1][:], start=True, stop=True)
            else:
                nc.tensor.matmul(ps[:], M4t[:], Zs[tau - 4][:], start=True, stop=True)
            Z = chain.tile([128, SD], f32, tag="Z")
            if tau % 2 == 0:
                nc.vector.tensor_copy(Z[:], ps[:])
            else:
                nc.scalar.copy(Z[:], ps[:])
            Zs[tau] = Z
            if tau < T:
                extract(tau, Z)
            if (tau + 1) % TPB == 0 and (tau + 1) // TPB < NB:
                emit_block_reshape((tau + 1) // TPB - 1)
            if tau == T // 2:
                emit_g_half(1)   # G rows T/2..T-1 came from chain steps 0..T/2-1
        emit_block_reshape(NB - 1, post=True)
        emit_g_half(0, post=True)

        # final frame x_T from the last chain state
        ps_xT = psT.tile([NL, D], f32, tag="pst")
        nc.tensor.transpose(ps_xT[:], Zs[T][0:D, 0:NL], idA)
        xT_sb = osb.tile([NL, D], f32, tag="xT_sb")
        nc.scalar.copy(xT_sb[:], ps_xT[:])
        nc.sync.dma_start(traj_out[NF - 1], xT_sb[:])

        # ---- big stage: paired even/odd chunks in concurrent row groups ----
        jac_pair = jac_out.rearrange("n (gq p2) j -> gq n (p2 j)", p2=2)
        m = 0
        for g in range(NG):
            for h in range(NH):
                hs = slice(h * 512, (h + 1) * 512)
                ps = psB.tile([128, 1024], f32)
                nc.tensor.matmul(
                    ps[:, 0:512],
                    GfatA[0:K, g * 256:(g + 1) * 256:2],
                    XfatA[0:K, hs],
                    start=True, stop=True,
                )
                nc.tensor.matmul(
                    ps[:, 512:1024],
                    GfatB[K:2 * K, g * 256 + 1:(g + 1) * 256:2],
                    XfatB[K:2 * K, hs],
                    start=True, stop=True,
                    tile_position=(64, 0),
                )
                ot = osb.tile([128, 1024], f32, tag="ot")
                ov = ot[:].rearrange("q (n p2 j) -> q n p2 j", n=NPH, p2=2)
                iv = ps[:].rearrange("q (p2 n j) -> q n p2 j", p2=2, n=NPH)
                if m % 3 == 2:
                    nc.scalar.copy(ov, iv)
                else:
                    nc.vector.tensor_copy(ov, iv)
                nc.sync.dma_start(
                    jac_pair[g * 128:(g + 1) * 128, h * NPH:(h + 1) * NPH, :],
                    ot[:].rearrange("q (n p2 j) -> q n (p2 j)", n=NPH, p2=2),
                )
                m += 1

    nc.compile()
    return nc


_cache = {}


def _get_nc(T, SE, NL):
    key = (T, SE, NL)
    if key not in _cache:
        _cache[key] = _build_nc(T, SE, NL)
    return _cache[key]


def kernel(x0, v0, force_weight, n_steps, store_every, _trace=False):
    from concourse.bass_utils import run_bass_kernel_spmd

    x0 = np.asarray(x0, dtype=np.float32)
    v0 = np.asarray(v0, dtype=np.float32)
    W = np.asarray(force_weight, dtype=np.float32)
    T = int(n_steps)
    SE = int(store_every)
    n, d = x0.shape
    assert (n, d) == (N, D)
    NL = n // NCORES
    NF = T // SE + 1

    nc = _get_nc(T, SE, NL)
    in_maps = [
        {
            "x0t": np.ascontiguousarray(x0[k * NL:(k + 1) * NL].T),
            "v0t": np.ascontiguousarray(v0[k * NL:(k + 1) * NL].T),
            "wt": np.ascontiguousarray(W.T),
            "ident": np.eye(128, dtype=np.float32),
        }
        for k in range(NCORES)
    ]
    res = run_bass_kernel_spmd(
        nc, in_maps, core_ids=list(range(NCORES)), trace=_trace
    )
    kernel.last_results = res

    jac = np.concatenate(
        [res.results[k]["jac"].reshape(NL, D, D, D) for k in range(NCORES)], axis=0
    )
    traj = np.concatenate(
        [res.results[k]["traj"] for k in range(NCORES)], axis=1
    )
    assert traj.shape == (NF, n, d) and jac.shape == (n, d, D, D)
    return traj, jac
